# revision 13
# baseline (speedup 1.0000x reference)
"""Trainium2 Bass kernel for nn_CorrelatedAttentionBlock_81286551044296.

Shapes (hardcoded): x (4, 256, 512, 64) f32; Wq/Wk/Wv/Wo (256,256); b* (256,);
log_tau (1,).  8 NeuronCores, sharded over (batch b, F-half): core = b*2 + fh,
each core handles x[b, :, :, fh*32:(fh+1)*32] -> out same slice.  Fully
independent shards (no collectives).

Algorithm per (core, f):  let X = x[b,:,:,f]  (C=256 x T=512)
  G    = X X^T                 (Gram over time)           [PE fp8 DoubleRow]
  A|Ak = (G/16) @ [Wq^T|Wk^T]                             [PE fp8 DR]
  prod = wqk8 .* (A|Ak);  ss[e] = colsum (ones-DR)        [DVE + PE]
  rs   = sqrt(inv_tau * recip(ss))   (pair-batched)       [DVE + ACT]
  cov  = Wk (G/16) Wq^T  diag 128-blocks                  [PE fp8 DR]
  scl  = outer(rs_k, rs_q) (K=1 matmuls)                  [PE]
  e    = exp(cov*scl) .* blockmask(32-seg)                [DVE + ACT + DVE]
  att  = e * recip(rowsum(e))                             [DVE + ACT]
  U^T  = att^T-mix of Wv ;  Z = U Wo^T                    [PE bf16]
  out^T = Z^T X  (bf16 out, bias bo added on host)        [PE bf16]

fp8 (e4m3) only on the G->cov/ss path where errors average out in bilinear
forms and cancel in softmax normalization; the V-side (U, Z, final) stays
bf16 since its errors hit the output directly.  The uniform 1/16 G-scale
cancels between cov and the rs normalizers.  f's are processed in pairs so
the small recip/sqrt ops amortize over two frequencies (rows at partition
0 and 32 of a shared tile, replicated x32 by the ones-matmul so no
uninitialized PSUM lanes are read).
"""

import numpy as np
import ml_dtypes

B, C, T, FQ = 4, 256, 512, 64
H, DH = 8, 32
FL = FQ // 2  # 32 f per core
N_CORES = 8

BF16 = ml_dtypes.bfloat16
F8 = ml_dtypes.float8_e4m3

_PROGRAM_CACHE = {}


def _build_program(inv_tau: float, reps: int = 1):
    import concourse.bacc as bacc
    import concourse.tile as tile
    from concourse import mybir
    from contextlib import ExitStack

    dt = mybir.dt
    AF = mybir.ActivationFunctionType
    DR = mybir.MatmulPerfMode.DoubleRow
    AX = mybir.AxisListType
    ALU = mybir.AluOpType

    nc = bacc.Bacc()
    x_tm = nc.declare_dram_parameter("x_tm", [FL, 128, 4, 256], dt.float8e4, isOutput=False)
    x_cm = nc.declare_dram_parameter("x_cm", [FL, 128, 2, 512], dt.bfloat16, isOutput=False)
    wqk8 = nc.declare_dram_parameter("wqk8", [128, 2, 512], dt.float8e4, isOutput=False)
    wv = nc.declare_dram_parameter("wv", [128, 2, 256], dt.bfloat16, isOutput=False)
    wo = nc.declare_dram_parameter("wo", [128, 2, 256], dt.bfloat16, isOutput=False)
    bmask = nc.declare_dram_parameter("bmask", [128, 2, 128], dt.bfloat16, isOutput=False)
    ones8 = nc.declare_dram_parameter("ones8", [128, 2, 32], dt.float8e4, isOutput=False)
    out_d = nc.declare_dram_parameter("out_d", [FL, 128, 2, 512], dt.bfloat16, isOutput=True)

    with tile.TileContext(nc) as tc:
        with ExitStack() as ctx:
            wpool = ctx.enter_context(tc.tile_pool(name="w", bufs=1))
            xpool = ctx.enter_context(tc.tile_pool(name="x", bufs=4))
            spool = ctx.enter_context(tc.tile_pool(name="s", bufs=3))
            opool = ctx.enter_context(tc.tile_pool(name="o", bufs=3))
            # PSUM: 8 banks x 2KB/partition -- budgeted exactly:
            # pg(G)=1, pa(aak,o)=2, pss=2, pcv(cov+scl)=1, puz(ut+z tags)=2
            pg = ctx.enter_context(tc.tile_pool(name="pg", bufs=1, space="PSUM"))
            pa = ctx.enter_context(tc.tile_pool(name="pa", bufs=2, space="PSUM"))
            pss = ctx.enter_context(tc.tile_pool(name="ps", bufs=1, space="PSUM"))
            pcv = ctx.enter_context(tc.tile_pool(name="pc", bufs=1, space="PSUM"))
            puz = ctx.enter_context(tc.tile_pool(name="pu", bufs=1, space="PSUM"))

            wqk_t = wpool.tile([128, 2, 512], dt.float8e4)
            nc.sync.dma_start(wqk_t[:], wqk8[:])
            wv_t = wpool.tile([128, 2, 256], dt.bfloat16)
            nc.sync.dma_start(wv_t[:], wv[:])
            wo_t = wpool.tile([128, 2, 256], dt.bfloat16)
            nc.sync.dma_start(wo_t[:], wo[:])
            bm_t = wpool.tile([128, 2, 128], dt.bfloat16)
            nc.sync.dma_start(bm_t[:], bmask[:])
            on_t = wpool.tile([128, 2, 32], dt.float8e4)
            nc.sync.dma_start(on_t[:], ones8[:])

            n_iter = FL * reps
            assert n_iter % 2 == 0
            for pair in range(n_iter // 2):
                fs = [(2 * pair) % FL, (2 * pair + 1) % FL]
                # ---- phase 1 (per f): G, A|Ak, prod, ss ----
                xcm_t, a8_t = [], []
                # ss rows (x32 replicated, partition 0): per-f 512-col range
                ss_t = pss.tile([32, 2, 512], dt.float32, tag="ss")
                for j, f in enumerate(fs):
                    xtm = xpool.tile([128, 4, 256], dt.float8e4, tag="xtm")
                    nc.sync.dma_start(xtm[:], x_tm[f])
                    xcm = xpool.tile([128, 2, 512], dt.bfloat16, tag="xcm")
                    nc.sync.dma_start(xcm[:], x_cm[f])
                    xcm_t.append(xcm)

                    # G[i,j] = sum_t X[i,t] X[j,t]  (fp8 DR: K=256/instr)
                    g_ps = pg.tile([128, 2, 256], dt.float32, tag="g")
                    for ib in range(2):
                        for tp in range(2):
                            nc.tensor.matmul(
                                g_ps[:, ib, :],
                                lhsT=xtm[:, 2 * tp:2 * tp + 2, ib * 128:(ib + 1) * 128],
                                rhs=xtm[:, 2 * tp:2 * tp + 2, :],
                                start=(tp == 0), stop=(tp == 1),
                                perf_mode=DR,
                            )
                    # g8 = G/16 fp8 (scale cancels in softmax normalization)
                    g8 = spool.tile([128, 2, 256], dt.float8e4, tag="g8")
                    nc.scalar.activation(g8[:], g_ps[:], AF.Copy, scale=0.0625)

                    # A|Ak = (G/16) @ [WqT | WkT]   (fp8 DR)
                    aak8 = spool.tile([128, 2, 512], dt.float8e4, tag="a8")
                    a8_t.append(aak8)
                    prod8 = spool.tile([128, 2, 512], dt.float8e4, tag="prod")
                    for ic in range(2):
                        aak_ps = pa.tile([128, 512], dt.float32, tag="aak")
                        nc.tensor.matmul(
                            aak_ps[:],
                            lhsT=g8[:, :, ic * 128:(ic + 1) * 128],
                            rhs=wqk_t[:],
                            start=True, stop=True,
                            perf_mode=DR,
                        )
                        if ic == 0:
                            nc.scalar.copy(aak8[:, ic, :], aak_ps[:])
                        else:
                            nc.vector.tensor_copy(aak8[:, ic, :], aak_ps[:])
                    # prod = wqk8 .* aak8 (SBUF-only, GPSIMD)
                    nc.gpsimd.tensor_mul(prod8[:], wqk_t[:], aak8[:])

                    # ss cols [q0|q1|k0|k1]x128 for this f (x32 replicated)
                    for qk in range(2):
                        for b2 in range(2):
                            cols = slice(qk * 256 + b2 * 128, qk * 256 + (b2 + 1) * 128)
                            nc.tensor.matmul(
                                ss_t[:, j, cols],
                                lhsT=on_t[:],
                                rhs=prod8[:, :, cols],
                                start=True, stop=True,
                                perf_mode=DR,
                            )

                # ---- pair-shared: rs = sqrt(inv_tau / ss) ----
                rr = spool.tile([32, 2, 512], dt.float32, tag="rr")
                nc.vector.reciprocal(rr[:], ss_t[:])
                rs_t = spool.tile([32, 2, 512], dt.bfloat16, tag="rs")
                nc.scalar.activation(rs_t[:], rr[:], AF.Sqrt, scale=float(inv_tau))

                # ---- phase 2 (per f): cov, scl, softmax, U, Z, out ----
                for j, f in enumerate(fs):
                    aak8v = a8_t[j]
                    xcm = xcm_t[j]
                    # cov diag blocks (cols 0:2) + scl outers (cols 2:4)
                    cs_ps = pcv.tile([128, 4, 128], dt.float32, tag="cs")
                    for b2 in range(2):
                        nc.tensor.matmul(
                            cs_ps[:, b2, :],
                            lhsT=wqk_t[:, :, 256 + b2 * 128: 256 + (b2 + 1) * 128],
                            rhs=aak8v[:, :, b2 * 128:(b2 + 1) * 128],
                            start=True, stop=True,
                            perf_mode=DR,
                        )
                    for b2 in range(2):
                        nc.tensor.matmul(
                            cs_ps[:, 2 + b2, :],
                            lhsT=rs_t[0:1, j, 256 + b2 * 128: 256 + (b2 + 1) * 128],
                            rhs=rs_t[0:1, j, b2 * 128:(b2 + 1) * 128],
                            start=True, stop=True,
                        )

                    # scl -> SBUF (DVE can't read two PSUM operands in one TT)
                    scl_sb = spool.tile([128, 2, 128], dt.bfloat16, tag="scl")
                    nc.scalar.copy(scl_sb[:], cs_ps[:, 2:4, :])
                    # e = exp(cov*scl) .* blockmask ; att = e*recip(rowsum)
                    covt = spool.tile([128, 2, 128], dt.float32, tag="covt")
                    nc.vector.tensor_mul(covt[:], cs_ps[:, 0:2, :], scl_sb[:])
                    e_raw = spool.tile([128, 2, 128], dt.bfloat16, tag="eraw")
                    nc.scalar.activation(e_raw[:], covt[:], AF.Exp)
                    e_bf = spool.tile([128, 2, 128], dt.bfloat16, tag="ebf")
                    nc.gpsimd.tensor_mul(e_bf[:], e_raw[:], bm_t[:])
                    rsum = spool.tile([128, 2], dt.float32, tag="rsum")
                    nc.vector.tensor_reduce(rsum[:], e_bf[:], axis=AX.X, op=ALU.add)
                    rinv = spool.tile([128, 2], dt.float32, tag="rinv")
                    nc.vector.reciprocal(rinv[:], rsum[:])
                    att = spool.tile([128, 2, 128], dt.bfloat16, tag="att")
                    for b2 in range(2):
                        nc.gpsimd.tensor_scalar_mul(att[:, b2, :], e_bf[:, b2, :],
                                                    rinv[:, b2:b2 + 1])

                    # U^T[e,i] = sum_d att[d,e] Wv[d,i]
                    ut_ps = puz.tile([128, 2, 256], dt.float32, tag="ut")
                    for b2 in range(2):
                        nc.tensor.matmul(
                            ut_ps[:, b2, :],
                            lhsT=att[:, b2, :],
                            rhs=wv_t[:, b2, :],
                            start=True, stop=True,
                        )
                    ut_bf = spool.tile([128, 2, 256], dt.bfloat16, tag="utbf")
                    nc.scalar.copy(ut_bf[:], ut_ps[:])

                    # Z[i,c] = sum_e U[i,e] WoT[e,c]
                    z_ps = puz.tile([128, 2, 256], dt.float32, tag="z")
                    for ib in range(2):
                        for b2 in range(2):
                            nc.tensor.matmul(
                                z_ps[:, ib, :],
                                lhsT=ut_bf[:, b2, ib * 128:(ib + 1) * 128],
                                rhs=wo_t[:, b2, :],
                                start=(b2 == 0), stop=(b2 == 1),
                            )
                    z_bf = spool.tile([128, 2, 256], dt.bfloat16, tag="zbf")
                    nc.vector.tensor_copy(z_bf[:], z_ps[:])

                    # out^T[c,t] = sum_i Z[i,c] X[i,t]  (bias added on host)
                    fout = opool.tile([128, 2, 512], dt.bfloat16, tag="fout")
                    for cb in range(2):
                        o_ps = pa.tile([128, 512], dt.float32, tag="aak")
                        for ib in range(2):
                            nc.tensor.matmul(
                                o_ps[:],
                                lhsT=z_bf[:, ib, cb * 128:(cb + 1) * 128],
                                rhs=xcm[:, ib, :],
                                start=(ib == 0), stop=(ib == 1),
                            )
                        if cb == 0:
                            nc.scalar.copy(fout[:, cb, :], o_ps[:])
                        else:
                            nc.vector.tensor_copy(fout[:, cb, :], o_ps[:])
                    nc.sync.dma_start(out_d[f], fout[:])

    nc.finalize()
    return nc


def get_program(inv_tau: float):
    key = round(float(inv_tau), 12)
    if key not in _PROGRAM_CACHE:
        _PROGRAM_CACHE[key] = _build_program(inv_tau)
    return _PROGRAM_CACHE[key]


def make_core_inputs(x, Wq, Wk, Wv, Wo, bo):
    """Host-side shard + layout prep. Returns list of 8 in_maps."""
    wqk8 = np.concatenate([Wq.T, Wk.T], axis=1).astype(F8)  # (256, 512)
    wqk8 = wqk8.reshape(2, 128, 512).transpose(1, 0, 2).copy()
    wv = Wv.astype(BF16).reshape(2, 128, 256).transpose(1, 0, 2).copy()
    wo = Wo.T.astype(BF16).reshape(2, 128, 256).transpose(1, 0, 2).copy()
    # multiplicative mask: 1 on same 32-head segment, 0 off
    p = np.arange(128)[:, None] // 32
    e = np.arange(128)[None, :] // 32
    blk = np.where(p == e, 1.0, 0.0).astype(BF16)
    bmask = np.stack([blk, blk], axis=1).copy()  # [128, 2, 128]
    ones8 = np.ones((128, 2, 32), F8)

    in_maps = []
    for core in range(N_CORES):
        b, fh = divmod(core, 2)
        xs = x[b, :, :, fh * FL:(fh + 1) * FL]  # (C, T, FL)
        xtm = np.ascontiguousarray(xs.transpose(2, 1, 0)).astype(F8)  # (FL, T, C)
        xtm = xtm.reshape(FL, 4, 128, 256).transpose(0, 2, 1, 3).copy()
        xcm = np.ascontiguousarray(xs.transpose(2, 0, 1)).astype(BF16)  # (FL, C, T)
        xcm = xcm.reshape(FL, 2, 128, 512).transpose(0, 2, 1, 3).copy()
        in_maps.append({
            "x_tm": xtm, "x_cm": xcm, "wqk8": wqk8, "wv": wv, "wo": wo,
            "bmask": bmask, "ones8": ones8,
        })
    return in_maps


def assemble_output(results):
    out = np.empty((B, C, T, FQ), dtype=np.float32)
    for core in range(N_CORES):
        b, fh = divmod(core, 2)
        od = np.asarray(results[core]["out_d"], dtype=np.float32)  # (FL,128,2,512)
        # od[f, p, cb, t] = out[b, cb*128+p, t, f0+f]
        o = od.transpose(2, 1, 3, 0).reshape(C, T, FL)
        out[b, :, :, fh * FL:(fh + 1) * FL] = o
    return out


def _numpy_reference(x, Wq, bq, Wk, bk, Wv, bv, Wo, bo, log_tau):
    xf = np.transpose(x, (0, 2, 3, 1)).astype(np.float64)

    def split(t):
        return np.transpose(t.reshape(B, T, FQ, H, DH), (0, 3, 1, 2, 4))

    Qh = split(xf @ Wq.T.astype(np.float64) + bq)
    Kh = split(xf @ Wk.T.astype(np.float64) + bk)
    Vh = split(xf @ Wv.T.astype(np.float64) + bv)

    def l2t(a, eps=1e-8):
        return a / np.sqrt(np.clip(np.sum(a * a, axis=2, keepdims=True), eps, None))

    Qh, Kh = l2t(Qh), l2t(Kh)
    tau = np.clip(np.exp(log_tau.astype(np.float64)), 1e-4, 10.0)
    cov = np.einsum('bhtfd,bhtfe->bhfde', Kh, Qh) / tau
    cov = cov - cov.max(axis=-1, keepdims=True)
    ecov = np.exp(cov)
    att = ecov / ecov.sum(axis=-1, keepdims=True)
    out_h = np.einsum('bhtfd,bhfde->bhtfe', Vh, att)
    out_tf = np.transpose(out_h, (0, 2, 3, 1, 4)).reshape(B, T, FQ, C)
    out_tf = out_tf @ Wo.T.astype(np.float64) + bo
    return np.transpose(out_tf, (0, 3, 1, 2)).astype(np.float32)


def kernel(x, Wq, bq, Wk, bk, Wv, bv, Wo, bo, log_tau):
    x = np.asarray(x, dtype=np.float32)
    Wq, Wk, Wv, Wo = (np.asarray(w, dtype=np.float32) for w in (Wq, Wk, Wv, Wo))
    bq, bk, bv, bo = (np.asarray(v, dtype=np.float32) for v in (bq, bk, bv, bo))
    log_tau = np.asarray(log_tau, dtype=np.float32)

    if np.any(bq) or np.any(bk) or np.any(bv):
        # general-case fallback (never hit for this problem's inputs)
        return _numpy_reference(x, Wq, bq, Wk, bk, Wv, bv, Wo, bo, log_tau)

    from concourse.bass_utils import run_bass_kernel_spmd

    tau = float(np.clip(np.exp(log_tau[0]), 1e-4, 10.0))
    nc = get_program(1.0 / tau)
    in_maps = make_core_inputs(x, Wq, Wk, Wv, Wo, bo)
    res = run_bass_kernel_spmd(nc, in_maps, list(range(N_CORES)))
    out = assemble_output(res.results)
    if np.any(bo):
        out += bo[None, :, None, None]
    return out
